# revision 3
# baseline (speedup 1.0000x reference)
"""Binarized 3x3 conv (BConv) Trainium2 Bass kernel — 1-D Winograd F(2,3).

Problem: x[32,256,56,56] f32, W[256,256,3,3] f32.
  out = conv2d(x, sign(W), stride 1, pad 1)  (NCHW / OIHW)

Strategy:
  - Data-parallel over batch: 8 cores x 4 images each, identical SPMD program.
  - Host casts x/W to bf16 and the device returns bf16 (upcast on host):
    halves all HBM traffic and removes on-chip f32->bf16 cast work.
  - Winograd F(2,3) along H only: per pair of output rows, the 3 h-taps
    of the conv collapse into 4 transform-domain products Yt[u], u=0..3
    (4 muls per 2 outputs instead of 6) -> PE time x(2/3).
      Xt[0]=t0-t2  Xt[1]=t1+t2  Xt[2]=t2-t1  Xt[3]=t1-t3   (t = 4 padded rows)
      Yt[u] = sum_kw sum_ci Wt[u,kw]^T Xt[u] (shifted by kw)   [PSUM, 6 steps]
      y[2i]   = Yt[0]+Yt[1]+Yt[2]                              [DVE]
      y[2i+1] = Yt[1]-Yt[2]-Yt[3]                              [DVE]
    Wt[u] = (G sign(W))_u / 2 == {u0=s0, u1=(s0+s1+s2)/2, u2=(s0-s1+s2)/2,
    u3=s2}/2 -- exact in bf16; the /2 is undone by the ACT PSUM-eviction
    scale (x2).
  - Input row-transform on DVE reads the DMA'd bf16 image directly with
    stride-2 row slices (contiguous rows -> 2x DVE mode); W-padding is
    materialized only in the 58-wide Xt buffer (left/right zero cols).
  - Weight prep on GpSimd keeps the DVE free for image 0's transform.
"""

import sys
from contextlib import ExitStack

sys.path.insert(0, "/opt/trn_rl_repo")

import numpy as np
import ml_dtypes

import concourse.mybir as mybir
import concourse.tile as tile
from concourse import bacc
from concourse.bass_utils import run_bass_kernel_spmd

N_CORES = 8
NIMG = 4          # images per core (32 / 8)
C = 256           # channels (in == out)
H = 56
P = 128           # partitions
NI = 28           # output row-pairs per image (56/2)
XW = 60           # Xt row pitch (58 logical cols + alignment pad)
IBS = (8, 8, 8, 4)  # row-pair blocks per image (sum 28); N = ib*56 <= 448

F32 = mybir.dt.float32
BF16 = mybir.dt.bfloat16
ALU = mybir.AluOpType

_cached = {}


def build_program():
    nc = bacc.Bacc("TRN2", target_bir_lowering=False, debug=False,
                   num_devices=N_CORES)

    x_d = nc.dram_tensor("x", [NIMG, C, H, H], BF16, kind="ExternalInput")
    # W arrives host-permuted to [C_in, kh, kw, C_out] bf16
    w_d = nc.dram_tensor("W", [C, 3, 3, C], BF16, kind="ExternalInput")
    y_d = nc.dram_tensor("y", [NIMG, C, H, H], BF16, kind="ExternalOutput")

    with tile.TileContext(nc) as tc, ExitStack() as ctx:
        wst_pool = ctx.enter_context(tc.tile_pool(name="wst", bufs=2))
        wsg_pool = ctx.enter_context(tc.tile_pool(name="wsg", bufs=4))
        stage_pool = ctx.enter_context(tc.tile_pool(name="stage", bufs=3))
        xt_pool = ctx.enter_context(tc.tile_pool(name="xt", bufs=4))
        yt_pool = ctx.enter_context(tc.tile_pool(name="yt", bufs=3))
        tmp_pool = ctx.enter_context(tc.tile_pool(name="tmp", bufs=4))
        out_pool = ctx.enter_context(tc.tile_pool(name="osb", bufs=4))
        psum_pool = ctx.enter_context(tc.tile_pool(name="ps", bufs=8,
                                                   space="PSUM"))

        # ---- weight prep (GpSimd; DVE stays free for image transforms) ----
        # wt[u][ih] access patterns producing Wt[u,kw][128ci, 128co] slices
        wu = [[None] * 2 for _ in range(4)]

        def prep_weights():
            for ih in range(2):
                wst = wst_pool.tile([P, 3, 3, C], BF16, tag="wst",
                                    name=f"wst_{ih}")
                nc.sync.dma_start(wst[:], w_d[ih * P:(ih + 1) * P])
                sa = wsg_pool.tile([P, 3, 3, C], BF16, tag="sa",
                                   name=f"sa_{ih}")
                # sa = sign(w)/2 = (w>=0) - 0.5
                nc.gpsimd.tensor_scalar(sa[:], wst[:], 0.0, 0.5,
                                        ALU.is_ge, ALU.subtract)
                sb = wsg_pool.tile([P, 3, 3, C], BF16, tag="sb",
                                   name=f"sb_{ih}")
                nc.gpsimd.tensor_scalar_mul(sb[:], sa[:], 0.5)
                u12 = wsg_pool.tile([P, 2, 3, C], BF16, tag="u12",
                                    name=f"u12_{ih}")
                t12 = wsg_pool.tile([P, 2, 3, C], BF16, tag="t12",
                                    name=f"t12_{ih}")
                nc.gpsimd.tensor_add(t12[:, 0], sb[:, 0], sb[:, 1])
                nc.gpsimd.tensor_sub(t12[:, 1], sb[:, 0], sb[:, 1])
                nc.gpsimd.tensor_add(u12[:, 0], t12[:, 0], sb[:, 2])
                nc.gpsimd.tensor_add(u12[:, 1], t12[:, 1], sb[:, 2])
                wu[0][ih] = sa[:, 0]    # [128, 3kw, 256co]
                wu[1][ih] = u12[:, 0]
                wu[2][ih] = u12[:, 1]
                wu[3][ih] = sa[:, 2]

        # ---- input stage + row transform for one (img, ihalf) ----
        def load_transform(img, ih, first=False):
            stg = stage_pool.tile([P, H, H], BF16, tag="stage",
                                  name=f"stage_{img}_{ih}")
            n_pieces = 2 if first else 1
            step_h = H // n_pieces
            for p_i in range(n_pieces):
                h0, h1 = p_i * step_h, (p_i + 1) * step_h
                nc.sync.dma_start(stg[:, h0:h1, :],
                                  x_d[img, ih * P:(ih + 1) * P, h0:h1])
            xt = xt_pool.tile([P, 4, NI, XW], BF16, tag="xt",
                              name=f"xt_{img}_{ih}")
            # zero the W-pad columns (logical col 0 -> phys 1, col 57 -> 58)
            nc.gpsimd.memset(xt[:, :, :, 1], 0.0)
            nc.gpsimd.memset(xt[:, :, :, 58], 0.0)
            d = xt[:, :, :, 2:58]   # logical cols 1..56 = x cols 0..55
            # tile i rows (padded) 2i..2i+3 = x rows 2i-1..2i+2
            # u0 = t0-t2 = x[2i-1]-x[2i+1]; u1 = t1+t2; u2 = t2-t1; u3 = t1-t3
            nc.vector.tensor_add(d[:, 1], stg[:, 0::2, :], stg[:, 1::2, :])
            nc.vector.tensor_sub(d[:, 2], stg[:, 1::2, :], stg[:, 0::2, :])
            nc.vector.tensor_scalar_mul(d[:, 0, 0], stg[:, 1, :], -1.0)
            nc.vector.tensor_sub(d[:, 0, 1:NI], stg[:, 1:54:2, :],
                                 stg[:, 3:56:2, :])
            nc.vector.tensor_sub(d[:, 3, 0:NI - 1], stg[:, 0:53:2, :],
                                 stg[:, 2:55:2, :])
            nc.vector.tensor_copy(d[:, 3, NI - 1], stg[:, 54, :])
            return xt

        # ---- conv for one (img, couth): 4 iblocks x 4 u-psums x 6 steps ----
        def conv_group(img, oc, xts):
            i0 = 0
            for ib in IBS:
                psums = [psum_pool.tile([P, 8, H], F32, tag="ps",
                                        name=f"ps_{img}_{oc}_{i0}_{u}")
                         for u in range(4)]
                for u in range(4):
                    step = 0
                    for ih in range(2):
                        for kw in range(3):
                            nc.tensor.matmul(
                                psums[u][:, :ib, :],
                                wu[u][ih][:, kw, oc * P:(oc + 1) * P],
                                xts[ih][:, u, i0:i0 + ib, 1 + kw:57 + kw],
                                start=(step == 0),
                                stop=(step == 5),
                            )
                            step += 1
                yt = yt_pool.tile([P, 4, 8, H], BF16, tag="yt",
                                  name=f"yt_{img}_{oc}_{i0}")
                for u in range(4):
                    nc.scalar.mul(yt[:, u, :ib, :], psums[u][:, :ib, :], 2.0)
                tmp = tmp_pool.tile([P, 2, 8, H], BF16, tag="tmp",
                                    name=f"tmp_{img}_{oc}_{i0}")
                osb = out_pool.tile([P, 8, 2, H], BF16, tag="osb",
                                    name=f"osb_{img}_{oc}_{i0}")
                nc.vector.tensor_add(tmp[:, 0, :ib], yt[:, 0, :ib], yt[:, 1, :ib])
                nc.vector.tensor_add(osb[:, :ib, 0], tmp[:, 0, :ib], yt[:, 2, :ib])
                nc.vector.tensor_sub(tmp[:, 1, :ib], yt[:, 1, :ib], yt[:, 2, :ib])
                nc.vector.tensor_sub(osb[:, :ib, 1], tmp[:, 1, :ib], yt[:, 3, :ib])
                nc.scalar.dma_start(
                    y_d[img, oc * P:(oc + 1) * P, 2 * i0:2 * (i0 + ib), :],
                    osb[:, :ib],
                )
                i0 += ib

        # ---- program order ----
        prep_weights()
        x0 = [load_transform(0, 0, first=True), load_transform(0, 1)]
        conv_group(0, 0, x0)
        x1 = [load_transform(1, 0), load_transform(1, 1)]
        conv_group(0, 1, x0)
        conv_group(1, 0, x1)
        x2 = [load_transform(2, 0), load_transform(2, 1)]
        conv_group(1, 1, x1)
        conv_group(2, 0, x2)
        x3 = [load_transform(3, 0), load_transform(3, 1)]
        conv_group(2, 1, x2)
        conv_group(3, 0, x3)
        conv_group(3, 1, x3)

    nc.compile()
    return nc


def _get_program():
    if "nc" not in _cached:
        _cached["nc"] = build_program()
    return _cached["nc"]


def kernel(x: np.ndarray, W: np.ndarray, trace: bool = False, **trace_kw):
    nc = _get_program()
    bf = ml_dtypes.bfloat16
    x = np.ascontiguousarray(np.asarray(x, dtype=np.float32).astype(bf))
    # host-side layout permutation only: [o,i,kh,kw] -> [i, kh, kw, o]
    w_r = np.ascontiguousarray(
        np.asarray(W, dtype=np.float32).transpose(1, 2, 3, 0).astype(bf))
    in_maps = [{"x": x[i * NIMG:(i + 1) * NIMG], "W": w_r}
               for i in range(N_CORES)]
    res = run_bass_kernel_spmd(nc, in_maps, core_ids=list(range(N_CORES)),
                               trace=trace, **trace_kw)
    out = np.concatenate(
        [np.asarray(res.results[i]["y"]).astype(np.float32)
         for i in range(N_CORES)], axis=0)
    if trace:
        return out, res
    return out


# revision 5
# speedup vs baseline: 2.1934x; 2.1934x over previous
"""Binarized 3x3 conv (BConv) Trainium2 Bass kernel — 1-D Winograd F(2,3).

Problem: x[32,256,56,56] f32, W[256,256,3,3] f32.
  out = conv2d(x, sign(W), stride 1, pad 1)  (NCHW / OIHW)

Strategy:
  - Data-parallel over batch: 8 cores x 4 images each, identical SPMD program.
  - Host casts x/W to bf16 and the device returns bf16 (upcast on host):
    halves all HBM traffic and removes on-chip f32->bf16 cast work.
  - Winograd F(2,3) along H only: per pair of output rows, the 3 h-taps
    of the conv collapse into 4 transform-domain products Yt[u], u=0..3
    (4 muls per 2 outputs instead of 6) -> PE time x(2/3).
      Xt[0]=t0-t2  Xt[1]=t1+t2  Xt[2]=t2-t1  Xt[3]=t1-t3   (t = 4 padded rows)
      Yt[u] = sum_kw sum_ci Wt[u,kw]^T Xt[u] (shifted by kw)   [PSUM, 6 steps]
      y[2i]   = Yt[0]+Yt[1]+Yt[2]                              [DVE]
      y[2i+1] = Yt[1]-Yt[2]-Yt[3]                              [DVE]
    Wt[u] = (G sign(W))_u / 2 == {u0=s0, u1=(s0+s1+s2)/2, u2=(s0-s1+s2)/2,
    u3=s2}/2 -- exact in bf16; the /2 is undone by the ACT PSUM-eviction
    scale (x2).
  - Input row-transform on DVE reads the DMA'd bf16 image directly with
    stride-2 row slices (contiguous rows -> 2x DVE mode); W-padding is
    materialized only in the 58-wide Xt buffer (left/right zero cols).
  - Weight prep on GpSimd keeps the DVE free for image 0's transform.
"""

import sys
from contextlib import ExitStack

sys.path.insert(0, "/opt/trn_rl_repo")

import numpy as np
import ml_dtypes

import concourse.mybir as mybir
import concourse.tile as tile
from concourse import bacc
from concourse.bass_utils import run_bass_kernel_spmd

N_CORES = 8
NIMG = 4          # images per core (32 / 8)
C = 256           # channels (in == out)
H = 56
P = 128           # partitions
NI = 28           # output row-pairs per image (56/2)
XW = 60           # Xt row pitch (58 logical cols + alignment pad)
IBS = (8, 8, 8, 4)  # row-pair blocks per image (sum 28); N = ib*56 <= 448

F32 = mybir.dt.float32
BF16 = mybir.dt.bfloat16
ALU = mybir.AluOpType

_cached = {}


def build_program():
    nc = bacc.Bacc("TRN2", target_bir_lowering=False, debug=False,
                   num_devices=N_CORES)

    x_d = nc.dram_tensor("x", [NIMG, C, H, H], BF16, kind="ExternalInput")
    # W arrives host-permuted to [C_in, kh, kw, C_out] bf16
    w_d = nc.dram_tensor("W", [C, 3, 3, C], BF16, kind="ExternalInput")
    y_d = nc.dram_tensor("y", [NIMG, C, H, H], BF16, kind="ExternalOutput")

    with tile.TileContext(nc) as tc, ExitStack() as ctx:
        wst_pool = ctx.enter_context(tc.tile_pool(name="wst", bufs=2))
        wsg_pool = ctx.enter_context(tc.tile_pool(name="wsg", bufs=4))
        stage_pool = ctx.enter_context(tc.tile_pool(name="stage", bufs=3))
        xt_pool = ctx.enter_context(tc.tile_pool(name="xt", bufs=4))
        yt_pool = ctx.enter_context(tc.tile_pool(name="yt", bufs=3))
        tmp_pool = ctx.enter_context(tc.tile_pool(name="tmp", bufs=4))
        out_pool = ctx.enter_context(tc.tile_pool(name="osb", bufs=4))
        psum_pool = ctx.enter_context(tc.tile_pool(name="ps", bufs=8,
                                                   space="PSUM"))

        # ---- weight prep (DVE; GpSimd tensor ops are pathologically slow) ----
        # wt[u][ih] access patterns producing Wt[u,kw][128ci, 128co] slices
        wu = [[None] * 2 for _ in range(4)]

        def prep_weights():
            for ih in range(2):
                wst = wst_pool.tile([P, 3, 3, C], BF16, tag="wst",
                                    name=f"wst_{ih}")
                nc.sync.dma_start(wst[:], w_d[ih * P:(ih + 1) * P])
                sa = wsg_pool.tile([P, 3, 3, C], BF16, tag="sa",
                                   name=f"sa_{ih}")
                # sa = sign(w)/2 = (w>=0) - 0.5
                nc.vector.tensor_scalar(sa[:], wst[:], 0.0, 0.5,
                                        ALU.is_ge, ALU.subtract)
                sb = wsg_pool.tile([P, 3, 3, C], BF16, tag="sb",
                                   name=f"sb_{ih}")
                nc.vector.tensor_scalar_mul(sb[:], sa[:], 0.5)
                u12 = wsg_pool.tile([P, 2, 3, C], BF16, tag="u12",
                                    name=f"u12_{ih}")
                t12 = wsg_pool.tile([P, 2, 3, C], BF16, tag="t12",
                                    name=f"t12_{ih}")
                nc.vector.tensor_add(t12[:, 0], sb[:, 0], sb[:, 1])
                nc.vector.tensor_sub(t12[:, 1], sb[:, 0], sb[:, 1])
                nc.vector.tensor_add(u12[:, 0], t12[:, 0], sb[:, 2])
                nc.vector.tensor_add(u12[:, 1], t12[:, 1], sb[:, 2])
                wu[0][ih] = sa[:, 0]    # [128, 3kw, 256co]
                wu[1][ih] = u12[:, 0]
                wu[2][ih] = u12[:, 1]
                wu[3][ih] = sa[:, 2]

        # ---- input stage + row transform for one (img, ihalf) ----
        def load_transform(img, ih, first=False):
            stg = stage_pool.tile([P, H, H], BF16, tag="stage",
                                  name=f"stage_{img}_{ih}")
            n_pieces = 2 if first else 1
            step_h = H // n_pieces
            for p_i in range(n_pieces):
                h0, h1 = p_i * step_h, (p_i + 1) * step_h
                nc.sync.dma_start(stg[:, h0:h1, :],
                                  x_d[img, ih * P:(ih + 1) * P, h0:h1])
            xt = xt_pool.tile([P, 4, NI, XW], BF16, tag="xt",
                              name=f"xt_{img}_{ih}")
            # zero the W-pad columns (logical col 0 -> phys 1, col 57 -> 58)
            nc.gpsimd.memset(xt[:, :, :, 1], 0.0)
            nc.gpsimd.memset(xt[:, :, :, 58], 0.0)
            d = xt[:, :, :, 2:58]   # logical cols 1..56 = x cols 0..55
            # tile i rows (padded) 2i..2i+3 = x rows 2i-1..2i+2
            # u0 = t0-t2 = x[2i-1]-x[2i+1]; u1 = t1+t2; u2 = t2-t1; u3 = t1-t3
            nc.vector.tensor_add(d[:, 1], stg[:, 0::2, :], stg[:, 1::2, :])
            nc.vector.tensor_sub(d[:, 2], stg[:, 1::2, :], stg[:, 0::2, :])
            nc.vector.tensor_scalar_mul(d[:, 0, 0], stg[:, 1, :], -1.0)
            nc.vector.tensor_sub(d[:, 0, 1:NI], stg[:, 1:54:2, :],
                                 stg[:, 3:56:2, :])
            nc.vector.tensor_sub(d[:, 3, 0:NI - 1], stg[:, 0:53:2, :],
                                 stg[:, 2:55:2, :])
            nc.vector.tensor_copy(d[:, 3, NI - 1], stg[:, 54, :])
            return xt

        # ---- conv for one (img, couth): iblock PAIRS, weight-stationary
        #      over the 2 blocks of a pair (2 matmuls per LDWEIGHTS),
        #      8 psum groups (4u x 2 blocks) in flight ----
        def conv_group(img, oc, xts):
            for pi, (ia, ibs) in enumerate((((0, 8), (8, 8)),
                                            ((16, 8), (24, 4)))):
                blocks = (ia, ibs)
                psums = [[psum_pool.tile([P, 8, H], F32, tag="ps",
                                         name=f"ps_{img}_{oc}_{pi}_{u}_{b}")
                          for b in range(2)] for u in range(4)]
                for u in range(4):
                    step = 0
                    for ih in range(2):
                        for kw in range(3):
                            for b, (i0, ib) in enumerate(blocks):
                                nc.tensor.matmul(
                                    psums[u][b][:, :ib, :],
                                    wu[u][ih][:, kw, oc * P:(oc + 1) * P],
                                    xts[ih][:, u, i0:i0 + ib, 1 + kw:57 + kw],
                                    start=(step == 0),
                                    stop=(step == 5),
                                )
                            step += 1
                for b, (i0, ib) in enumerate(blocks):
                    yt = yt_pool.tile([P, 4, 8, H], BF16, tag="yt",
                                      name=f"yt_{img}_{oc}_{i0}")
                    for u in range(4):
                        nc.scalar.mul(yt[:, u, :ib, :],
                                      psums[u][b][:, :ib, :], 2.0)
                    tmp = tmp_pool.tile([P, 2, 8, H], BF16, tag="tmp",
                                        name=f"tmp_{img}_{oc}_{i0}")
                    osb = out_pool.tile([P, 8, 2, H], BF16, tag="osb",
                                        name=f"osb_{img}_{oc}_{i0}")
                    nc.vector.tensor_add(tmp[:, 0, :ib], yt[:, 0, :ib],
                                         yt[:, 1, :ib])
                    nc.vector.tensor_add(osb[:, :ib, 0], tmp[:, 0, :ib],
                                         yt[:, 2, :ib])
                    nc.vector.tensor_sub(tmp[:, 1, :ib], yt[:, 1, :ib],
                                         yt[:, 2, :ib])
                    nc.vector.tensor_sub(osb[:, :ib, 1], tmp[:, 1, :ib],
                                         yt[:, 3, :ib])
                    nc.scalar.dma_start(
                        y_d[img, oc * P:(oc + 1) * P, 2 * i0:2 * (i0 + ib), :],
                        osb[:, :ib],
                    )

        # ---- program order ----
        prep_weights()
        x0 = [load_transform(0, 0, first=True), load_transform(0, 1)]
        conv_group(0, 0, x0)
        x1 = [load_transform(1, 0), load_transform(1, 1)]
        conv_group(0, 1, x0)
        conv_group(1, 0, x1)
        x2 = [load_transform(2, 0), load_transform(2, 1)]
        conv_group(1, 1, x1)
        conv_group(2, 0, x2)
        x3 = [load_transform(3, 0), load_transform(3, 1)]
        conv_group(2, 1, x2)
        conv_group(3, 0, x3)
        conv_group(3, 1, x3)

    nc.compile()
    return nc


def _get_program():
    if "nc" not in _cached:
        _cached["nc"] = build_program()
    return _cached["nc"]


def kernel(x: np.ndarray, W: np.ndarray, trace: bool = False, **trace_kw):
    nc = _get_program()
    bf = ml_dtypes.bfloat16
    x = np.ascontiguousarray(np.asarray(x, dtype=np.float32).astype(bf))
    # host-side layout permutation only: [o,i,kh,kw] -> [i, kh, kw, o]
    w_r = np.ascontiguousarray(
        np.asarray(W, dtype=np.float32).transpose(1, 2, 3, 0).astype(bf))
    in_maps = [{"x": x[i * NIMG:(i + 1) * NIMG], "W": w_r}
               for i in range(N_CORES)]
    res = run_bass_kernel_spmd(nc, in_maps, core_ids=list(range(N_CORES)),
                               trace=trace, **trace_kw)
    out = np.concatenate(
        [np.asarray(res.results[i]["y"]).astype(np.float32)
         for i in range(N_CORES)], axis=0)
    if trace:
        return out, res
    return out
